# revision 41
# baseline (speedup 1.0000x reference)
"""MoE layer (top-2 of 8 experts, SwiGLU) on 8 Trainium2 NeuronCores.

Strategy: expert parallelism with overflow sharding. The router runs on the
host; tokens are gathered per expert and each core runs one expert's three
GEMMs (bf16 operands, fp32 PSUM) with weights resident in SBUF. The SPMD
program pads every core to the same token count, so the heaviest expert's
overflow (count - CA tokens) is peeled off into a "B segment" that all 8
cores execute cooperatively, each holding a 3-chunk H-slice of the heavy
expert's weights; the host sums the partial outputs. This drops the padded
count CA from max(count) to max(second_max, max - 128).

Device layout is feature-on-partition / token-on-free:
  G.T = Wg.T.T @ X.T   per (H-chunk, D-chunk) tile, accumulated over D
  U.T = W1.T.T @ X.T
  h   = silu(G) * U    (ScalarE silu, VectorE multiply, bf16 result)
  Y.T = W2.T.T @ h     accumulated over H-chunks

Weights are packed host-side into hk-major SBUF images ([128, 22*1024],
col block (hk, d) holding the 128x128 stationary operand), so each matrix
loads with ~6 large DMA triggers ordered so the first H-chunks land first
(the baseline's 150 small triggers serialized ~630ns apiece on the Sync
queue and starved the PE for the first ~25us). A block of dummy matmuls on
a zeroed tile runs while the first weights stream in, so the PE HAM clock
gate reaches 2.4GHz before the real GEMM stream begins.
"""

import numpy as np
import ml_dtypes

B, S, D = 2, 4096, 1024
E, H, TOPK = 8, 2736, 2
T = B * S
HP = 2816  # H padded to a multiple of 128
KD = D // 128  # 8 contraction chunks over D
KH = HP // 128  # 22 chunks over padded H
N_CORES = 8
CT = 512  # max token tile (free dim per matmul, one PSUM bank of fp32)
BT = 128  # overflow (B segment) token capacity
KB = 3  # H-chunks per core in the B segment (8*3 >= 22)

_BF16 = ml_dtypes.bfloat16

# hk-group column ranges for weight DMA arrival order (units of 128-chunks).
# Singles at the head: one wg hk chunk is 256KB (~0.7us) against ~3.4us of
# GEMM1/2 compute per hk, so the DMA front stays ahead of the PE from hk0
# on; coarser groups would make the PE wait out a multi-MB transfer.
_WG_GRPS = [(0, 1), (1, 2), (2, 3), (3, 4), (4, 5), (5, 6), (6, 7), (7, 8),
            (8, 10), (10, 13), (13, 17), (17, 22)]
_W2_GRPS = [(0, 8), (8, 15), (15, 22)]


def _install_drain_patch():
    """walrus in this image rejects any instruction carrying >1 sync wait
    ("Too many sync wait commands"). Split waits: every instruction keeps one
    wait; extra waits ride dedicated NoOps inserted just before it on the
    same engine. Applies to the Tile-lowered stream and to the tail drain."""
    import concourse.mybir as mybir
    import concourse.tile as tile
    from concourse.vector_clock import ScopedClock

    if getattr(tile.TileContext, "_drain_patch_installed", False):
        return

    _orig_lower = tile.TileContext._lower_ordered_insts

    def _split_lower(self, ordered):
        nc = self.nc
        for bb_name, insts in ordered.items():
            new = []
            for inst in insts:
                si = inst.sync_info
                ow = list(si.on_wait) if si is not None and si.on_wait else []
                if len(ow) > 1:
                    scopes = self._inst_to_scopes.get(inst.name, ())
                    for w in ow[:-1]:
                        nop = mybir.InstNoOp(
                            name=nc.get_next_instruction_name(),
                            engine=inst.engine,
                            ins=[],
                            outs=[],
                            sync_info=mybir.SyncInfo(on_wait=[w], on_update=[]),
                            bass_nofuse=True,
                        )
                        if scopes:
                            self._inst_to_scopes[nop.name] = scopes
                        new.append(nop)
                    ou = list(si.on_update) if si.on_update else []
                    inst.sync_info = mybir.SyncInfo(on_wait=[ow[-1]], on_update=ou)
                new.append(inst)
            ordered[bb_name] = new
        return _orig_lower(self, ordered)

    tile.TileContext._lower_ordered_insts = _split_lower

    def _patched(self, tick_clock, wait_clock):
        nc = self.nc
        nops = [nc.sync.nop(nofuse=True) for _ in range(30)]
        drain_inst = nc.sync.drain()
        wait_clock.add_sem_waits(
            drain_inst.ins, ScopedClock({None: tick_clock.global_clock})
        )
        si = drain_inst.ins.sync_info
        ow = list(si.on_wait) if si is not None and si.on_wait else []
        if len(ow) > 1:
            assert len(ow) <= 1 + len(nops), f"drain needs {len(ow)} waits"
            for i, w in enumerate(ow[:-1]):
                nops[i].ins.sync_info = mybir.SyncInfo(on_wait=[w], on_update=[])
            ou = list(si.on_update) if si.on_update else []
            drain_inst.ins.sync_info = mybir.SyncInfo(on_wait=[ow[-1]], on_update=ou)
        nc.all_engine_barrier()
        assert self.sems is not None
        popped = nc._tile_sem_poison_stack.pop()
        assert popped is self._sem_poison
        nc.clear_and_free_semaphores(list(self.sems.allocated().values()))
        nc.all_engine_barrier()

    tile.TileContext._drain_and_barrier = _patched
    tile.TileContext._drain_patch_installed = True


def _token_tiles(C):
    """First tile maxed at CT (slow hk cadence while weights stream in),
    remainder near-equal (multiples of 8, each <=CT)."""
    sizes = []
    if C > CT:
        sizes.append(CT)
        C_rest = C - CT
    else:
        C_rest = C
    n = -(-C_rest // CT)
    base = (C_rest // n) // 8 * 8
    rest = [base] * n
    extra = (C_rest - base * n) // 8
    for i in range(extra):
        rest[i] += 8
    # shrink the last tile (shorter final copy+DMA tail), growing earlier
    # ones up to CT; floor 264 so the tile never goes LDWEIGHTS-bound
    if n > 1:
        take = min(rest[-1] - 264, sum(CT - s for s in rest[:-1]))
        if take > 0:
            rest[-1] -= take
            for i in range(n - 1):
                add = min(take, CT - rest[i])
                rest[i] += add
                take -= add
    sizes += rest
    tiles = []
    t0 = 0
    for ct in sizes:
        tiles.append((t0, ct))
        t0 += ct
    assert t0 == sum(sizes) and sum(sizes) == (C if C <= CT else C), (C, sizes)
    return tiles


_PROGRAM_CACHE = {}


def _build_program(CA):
    """One SPMD program: expert FFN over CA (padded) tokens, plus a B
    segment: BT tokens of the heavy expert against a KB-chunk H-slice."""
    if CA in _PROGRAM_CACHE:
        return _PROGRAM_CACHE[CA]

    _install_drain_patch()
    import concourse.bass as bass
    import concourse.mybir as mybir
    import concourse.tile as tile

    bf16 = mybir.dt.bfloat16
    f32 = mybir.dt.float32

    tiles = _token_tiles(CA)
    ct_max = max(ct for _, ct in tiles)

    nc = bass.Bass()
    xT = nc.declare_dram_parameter("xT", [128, KD * CA], bf16, isOutput=False)
    wgT = nc.declare_dram_parameter("wgT", [128, KH * 1024], bf16, isOutput=False)
    w1T = nc.declare_dram_parameter("w1T", [128, KH * 1024], bf16, isOutput=False)
    w2T = nc.declare_dram_parameter("w2T", [128, KH * 1024], bf16, isOutput=False)
    xbT = nc.declare_dram_parameter("xbT", [128, KD * BT], bf16, isOutput=False)
    wgbT = nc.declare_dram_parameter("wgbT", [128, KB * 1024], bf16, isOutput=False)
    w1bT = nc.declare_dram_parameter("w1bT", [128, KB * 1024], bf16, isOutput=False)
    w2bT = nc.declare_dram_parameter("w2bT", [128, KB * 1024], bf16, isOutput=False)
    yT = nc.declare_dram_parameter("yT", [D, CA], f32, isOutput=True)
    ybT = nc.declare_dram_parameter("ybT", [128, KD * BT], f32, isOutput=True)

    with tile.TileContext(nc) as tc:
        with (
            tc.tile_pool(name="wpool", bufs=1) as wpool,
            tc.tile_pool(name="xpool", bufs=2) as xpool,
            tc.tile_pool(name="hpool", bufs=1) as hpool,
            tc.tile_pool(name="gpool", bufs=2) as gpool,
            tc.tile_pool(name="ypool", bufs=2) as ypool,
            tc.tile_pool(name="pg", bufs=2, space="PSUM") as pg,
            tc.tile_pool(name="pu", bufs=2, space="PSUM") as pu,
            tc.tile_pool(name="py", bufs=4, space="PSUM") as py,
        ):
            # --- PE warmup: dummy matmuls on a zeroed tile keep the PE busy
            # while the first weight DMAs stream in, so the HAM clock gate
            # releases (1.2 -> 2.4 GHz) near the start of the real stream.
            warm = wpool.tile([128, ct_max + 128], bf16, tag="warm")
            nc.vector.memset(warm[:], 0.0)
            for _ in range(18):
                warm_ps = pg.tile([128, ct_max], f32, tag="g_ps")
                nc.tensor.matmul(
                    warm_ps[:],
                    warm[:, ct_max:ct_max + 128],
                    warm[:, 0:ct_max],
                    start=True,
                    stop=True,
                )

            # --- resident weights + tile-0 tokens. DMA trigger issue costs
            # ~650ns apiece and serializes per engine queue, so the triggers
            # are spread across the hardware-DGE engines: wg (+x quarters)
            # on Sync, w1 (+x quarters) on Scalar; w2/B ride GpSimd later.
            # Groups are ordered so the first H-chunks land first and the
            # GEMM1/2 hk cadence stays behind the DMA front.
            t0_0, ct_0 = tiles[0]
            x_tiles = {}
            x0 = xpool.tile([128, KD * ct_0], bf16, tag="x")
            x_tiles[0] = x0
            wg_sb = wpool.tile([128, KH * 1024], bf16, tag="wg")
            w1_sb = wpool.tile([128, KH * 1024], bf16, tag="w1")
            w2_sb = wpool.tile([128, KH * 1024], bf16, tag="w2")
            # x0 rides as quarter-slices interleaved into both weight queues
            # (GpSimd's software-DGE path is too slow for the startup x): the
            # PE needs x quarters in d order and one wg/w1 chunk per ~3.4us.
            q = 2 * ct_0
            for gi, (k0, k1) in enumerate(_WG_GRPS):
                nc.sync.dma_start(
                    wg_sb[:, k0 * 1024:k1 * 1024], wgT[:, k0 * 1024:k1 * 1024]
                )
                if gi == 0:
                    nc.sync.dma_start(x0[:, 0:q], xT[:, 0:q])
            for gi, (k0, k1) in enumerate(_WG_GRPS):
                nc.scalar.dma_start(
                    w1_sb[:, k0 * 1024:k1 * 1024], w1T[:, k0 * 1024:k1 * 1024]
                )
                if gi == 0:
                    nc.scalar.dma_start(x0[:, 2 * q:3 * q], xT[:, 2 * q:3 * q])
                    nc.scalar.dma_start(x0[:, q:2 * q], xT[:, q:2 * q])
                    nc.scalar.dma_start(x0[:, 3 * q:4 * q], xT[:, 3 * q:4 * q])
            # B segment inputs + w2 are needed only late in the run; their
            # triggers are emitted mid-tile-0 behind a data gate (see
            # run_tile) so their transfers don't steal HBM bandwidth from
            # the wg/w1 stream the PE is chewing through.
            xb_sb = wpool.tile([128, KD * BT], bf16, tag="xb")
            wgb_sb = wpool.tile([128, KB * 1024], bf16, tag="wgb")
            w1b_sb = wpool.tile([128, KB * 1024], bf16, tag="w1b")
            w2b_sb = wpool.tile([128, KB * 1024], bf16, tag="w2b")
            yb_sb = wpool.tile([128, KD * BT], f32, tag="yb")
            gate_sb = wpool.tile([128, 8], bf16, tag="gate")
            gate_done = [False]

            def emit_gated_loads(h_s, ct):
                # The Tile scheduler floats dependency-free DMA triggers to
                # the front of the engine queue, so a queue-order gate is
                # not enough: stamp a tiny sliver of every late-load dst
                # from a tile that depends on tile-0's hk-6 output (~35us
                # in). The WAW overlap forces each DMA behind the stamp.
                gate_done[0] = True
                nc.gpsimd.tensor_copy(gate_sb[:], h_s[:, 6 * ct:6 * ct + 8])
                if len(tiles) > 1:
                    _, ct1 = tiles[1]
                    x_s1 = xpool.tile([128, KD * ct1], bf16, tag="x")
                    nc.gpsimd.tensor_copy(x_s1[:, 0:8], gate_sb[:])
                    nc.gpsimd.dma_start(
                        x_s1[:], xT[:, xoff[1]:xoff[1] + KD * ct1]
                    )
                    x_tiles[1] = x_s1
                for k0, k1 in _W2_GRPS:
                    nc.gpsimd.tensor_copy(
                        w2_sb[:, k0 * 1024:k0 * 1024 + 8], gate_sb[:]
                    )
                    nc.gpsimd.dma_start(
                        w2_sb[:, k0 * 1024:k1 * 1024],
                        w2T[:, k0 * 1024:k1 * 1024],
                    )
                # B inputs ride the fast hardware-DGE queues (idle once
                # wg/w1 are through) — B compute starts with tile 1.
                for eng, pairs in (
                    (nc.sync, ((xb_sb, xbT), (wgb_sb, wgbT))),
                    (nc.scalar, ((w1b_sb, w1bT), (w2b_sb, w2bT))),
                ):
                    for dst, src in pairs:
                        nc.gpsimd.tensor_copy(dst[:, 0:8], gate_sb[:, 0:8])
                        eng.dma_start(dst[:], src[:])

            xoff = [0]
            for _, ct in tiles:
                xoff.append(xoff[-1] + KD * ct)

            def load_x(t_idx):
                _, ct = tiles[t_idx]
                x_s = xpool.tile([128, KD * ct], bf16, tag="x")
                nc.gpsimd.dma_start(
                    x_s[:], xT[:, xoff[t_idx]:xoff[t_idx] + KD * ct]
                )
                x_tiles[t_idx] = x_s

            def run_tile(ti, b_gen=None):
                t0, ct = tiles[ti]
                x_s = x_tiles.pop(ti)

                h_s = hpool.tile([128, KH * ct], bf16, tag="h")
                for hk in range(KH):
                    if ti == 0 and hk == 8:
                        emit_gated_loads(h_s, ct)
                    g_ps = pg.tile([128, ct], f32, tag="g_ps")
                    u_ps = pu.tile([128, ct], f32, tag="u_ps")
                    # d-order follows the startup x-quarter arrival order
                    # (quarters 0 and 2 ride first on their queues)
                    for di, dd in enumerate((0, 1, 4, 5, 2, 3, 6, 7)):
                        co = hk * 1024 + dd * 128
                        nc.tensor.matmul(
                            g_ps[:],
                            wg_sb[:, co:co + 128],
                            x_s[:, dd * ct:(dd + 1) * ct],
                            start=(di == 0),
                            stop=(di == KD - 1),
                        )
                        nc.tensor.matmul(
                            u_ps[:],
                            w1_sb[:, co:co + 128],
                            x_s[:, dd * ct:(dd + 1) * ct],
                            start=(di == 0),
                            stop=(di == KD - 1),
                        )
                        if b_gen is not None and (hk * KD + di) % 2 == 0:
                            next(b_gen, None)
                    g_tmp = gpool.tile([128, ct], f32, tag="g")
                    nc.scalar.activation(
                        g_tmp[:], g_ps[:], mybir.ActivationFunctionType.Silu
                    )
                    nc.vector.tensor_mul(
                        h_s[:, hk * ct:(hk + 1) * ct], g_tmp[:], u_ps[:]
                    )

                if ti + 1 < len(tiles) and ti + 1 not in x_tiles:
                    load_x(ti + 1)
                last_tile = ti == len(tiles) - 1
                for dd in range(KD):
                    y_ps = py.tile([128, ct], f32, tag="y_ps")
                    for hk in range(KH):
                        co = hk * 1024 + dd * 128
                        nc.tensor.matmul(
                            y_ps[:],
                            w2_sb[:, co:co + 128],
                            h_s[:, hk * ct:(hk + 1) * ct],
                            start=(hk == 0),
                            stop=(hk == KH - 1),
                        )
                    y_sb = ypool.tile([128, ct], f32, tag="y")
                    nc.vector.tensor_copy(y_sb[:], y_ps[:])
                    if last_tile and dd == KD - 1:
                        # final output of the whole program: split across two
                        # DMA queues so the tail transfer halves in time
                        h1 = (ct // 2) // 8 * 8
                        nc.sync.dma_start(
                            yT[dd * 128:(dd + 1) * 128, t0:t0 + h1],
                            y_sb[:, 0:h1],
                        )
                        nc.scalar.dma_start(
                            yT[dd * 128:(dd + 1) * 128, t0 + h1:t0 + ct],
                            y_sb[:, h1:ct],
                        )
                    else:
                        nc.sync.dma_start(
                            yT[dd * 128:(dd + 1) * 128, t0:t0 + ct], y_sb[:]
                        )

            def b_segment_ops():
                """B-segment emission as a generator: one op per yield, to
                be interleaved between a wide A tile's matmul pairs so
                B's N=128 LDWEIGHTS (107ns > 53ns stream) hide under the
                A stream instead of rate-limiting the PE. PSUM comes from
                the py pool, idle during the A tile's GEMM1/2 phase."""
                hb = hpool.tile([128, KB * BT], bf16, tag="hb")
                for kb in range(KB):
                    g_ps = py.tile([128, BT], f32, tag="y_ps", name="bg_ps")
                    u_ps = py.tile([128, BT], f32, tag="y_ps", name="bu_ps")
                    for dd in range(KD):
                        co = kb * 1024 + dd * 128
                        nc.tensor.matmul(
                            g_ps[:],
                            wgb_sb[:, co:co + 128],
                            xb_sb[:, dd * BT:(dd + 1) * BT],
                            start=(dd == 0),
                            stop=(dd == KD - 1),
                        )
                        yield
                        nc.tensor.matmul(
                            u_ps[:],
                            w1b_sb[:, co:co + 128],
                            xb_sb[:, dd * BT:(dd + 1) * BT],
                            start=(dd == 0),
                            stop=(dd == KD - 1),
                        )
                        yield
                    g_tmp = gpool.tile([128, BT], f32, tag="g", name="bg_tmp")
                    nc.scalar.activation(
                        g_tmp[:], g_ps[:], mybir.ActivationFunctionType.Silu
                    )
                    nc.vector.tensor_mul(
                        hb[:, kb * BT:(kb + 1) * BT], g_tmp[:], u_ps[:]
                    )
                    yield
                for dd in range(KD):
                    y_ps = py.tile([128, BT], f32, tag="y_ps", name="by_ps")
                    for kb in range(KB):
                        co = kb * 1024 + dd * 128
                        nc.tensor.matmul(
                            y_ps[:],
                            w2b_sb[:, co:co + 128],
                            hb[:, kb * BT:(kb + 1) * BT],
                            start=(kb == 0),
                            stop=(kb == KB - 1),
                        )
                        yield
                    nc.vector.tensor_copy(
                        yb_sb[:, dd * BT:(dd + 1) * BT], y_ps[:]
                    )
                    yield
                nc.sync.dma_start(ybT[:], yb_sb[:])
                yield

            # interleave B into the second tile: its A matmuls stream long
            # enough to hide B's extra LDWEIGHTS, and the gated B inputs
            # have landed by the time tile 1 runs
            b_tile = 1 if len(tiles) > 1 else None
            for ti in range(len(tiles)):
                b_gen = b_segment_ops() if ti == b_tile else None
                run_tile(ti, b_gen)
                if b_gen is not None:
                    for _ in b_gen:  # drain any remainder
                        pass
            if b_tile is None:
                for _ in b_segment_ops():
                    pass

    _PROGRAM_CACHE[CA] = nc
    return nc


def _route(xf, gate_w):
    """Host router matching the reference: fp32 logits/softmax, top-2."""
    logits = xf @ gate_w.T  # [T, E] fp32
    m = logits.max(axis=1, keepdims=True)
    p = np.exp(logits - m, dtype=np.float32)
    p /= p.sum(axis=1, keepdims=True)
    # softmax is monotonic in logits, so top-2 by probs == top-2 by logits
    top_i = np.argsort(-p, axis=1, kind="stable")[:, :TOPK]  # [T, 2]
    top_p = np.take_along_axis(p, top_i, axis=1)
    gate_weights = top_p / (top_p.sum(axis=1, keepdims=True) + np.float32(1e-8))
    return top_i, gate_weights.astype(np.float32)


def _pack_x(rows_x, tiles):
    """[n, D] bf16 token block -> [128, KD*sum(ct)] tile-major image."""
    n = rows_x.shape[0]
    CA = sum(ct for _, ct in tiles)
    out = np.zeros((128, KD * CA), dtype=_BF16)
    off = 0
    for t0, ct in tiles:
        nv = max(0, min(ct, n - t0))
        if nv > 0:
            blk = rows_x[t0:t0 + nv].T.reshape(KD, 128, nv).transpose(1, 0, 2)
            dst = out[:, off:off + KD * ct].reshape(128, KD, ct)
            dst[:, :, :nv] = blk
        off += KD * ct
    return out


def _pack_wg(Wmat):
    """[H, D] -> [128, KH*1024]: col block (hk, d) = Wmat[hk*128+c, d*128+p]."""
    A = np.zeros((HP, D), dtype=_BF16)
    A[:H] = Wmat.astype(_BF16)
    return np.ascontiguousarray(
        A.reshape(KH, 128, KD, 128).transpose(3, 0, 2, 1).reshape(128, KH * 1024)
    )


def _pack_w2(Wmat):
    """[D, H] -> [128, KH*1024]: col block (hk, d) = Wmat[d*128+c, hk*128+p]."""
    A = np.zeros((D, HP), dtype=_BF16)
    A[:, :H] = Wmat.astype(_BF16)
    return np.ascontiguousarray(
        A.reshape(KD, 128, KH, 128).transpose(3, 2, 0, 1).reshape(128, KH * 1024)
    )


def kernel(x, gate_w, Wg, W1, W2):
    from concourse.bass_utils import run_bass_kernel_spmd

    x = np.asarray(x, dtype=np.float32)
    gate_w = np.asarray(gate_w, dtype=np.float32)
    Wg = np.asarray(Wg, dtype=np.float32)
    W1 = np.asarray(W1, dtype=np.float32)
    W2 = np.asarray(W2, dtype=np.float32)

    xf = x.reshape(-1, D)
    top_i, gate_weights = _route(xf, gate_w)

    idx = [None] * E
    wts = [None] * E
    for e in range(E):
        rows, slots = np.nonzero(top_i == e)
        idx[e] = rows
        wts[e] = gate_weights[rows, slots]
    counts = np.array([len(i) for i in idx])

    heavy = int(np.argmax(counts))
    second = int(np.max(np.delete(counts, heavy))) if E > 1 else 0
    CA = max(second, int(counts[heavy]) - BT, 128)
    CA = -(-CA // 8) * 8
    nb = max(0, int(counts[heavy]) - CA)  # heavy-expert overflow tokens
    assert nb <= BT, (counts, CA, nb)
    tiles = _token_tiles(CA)

    nc = _build_program(CA)

    xf_bf = xf.astype(_BF16)

    # B segment: overflow tokens of the heavy expert, H-sharded over cores
    b_rows = idx[heavy][CA:]
    xb_img = np.zeros((128, KD * BT), dtype=_BF16)
    if nb > 0:
        blk = xf_bf[b_rows].T.reshape(KD, 128, nb).transpose(1, 0, 2)
        xb_img.reshape(128, KD, BT)[:, :, :nb] = blk
    wg_heavy = _pack_wg(Wg[heavy])
    w1_heavy = _pack_wg(W1[heavy])
    w2_heavy = _pack_w2(W2[heavy])
    pad_cols = (N_CORES * KB - KH) * 1024
    wg_heavy_p = np.pad(wg_heavy, ((0, 0), (0, pad_cols)))
    w1_heavy_p = np.pad(w1_heavy, ((0, 0), (0, pad_cols)))
    w2_heavy_p = np.pad(w2_heavy, ((0, 0), (0, pad_cols)))

    in_maps = []
    for e in range(E):
        c0 = e * KB * 1024
        c1 = c0 + KB * 1024
        in_maps.append({
            "xT": _pack_x(xf_bf[idx[e][:CA]], tiles),
            "wgT": _pack_wg(Wg[e]),
            "w1T": _pack_wg(W1[e]),
            "w2T": _pack_w2(W2[e]),
            "xbT": xb_img,
            "wgbT": wg_heavy_p[:, c0:c1],
            "w1bT": w1_heavy_p[:, c0:c1],
            "w2bT": w2_heavy_p[:, c0:c1],
        })

    res = run_bass_kernel_spmd(nc, in_maps, list(range(N_CORES)))

    out = np.zeros((T, D), dtype=np.float32)
    for e in range(E):
        ne = min(int(counts[e]), CA)
        yT_e = res.results[e]["yT"]  # [D, CA] fp32
        out[idx[e][:ne]] += wts[e][:ne, None] * yT_e.T[:ne]
    if nb > 0:
        yb_sum = np.zeros((128, KD * BT), dtype=np.float32)
        for e in range(E):
            yb_sum += res.results[e]["ybT"]
        yb = yb_sum.reshape(128, KD, BT).transpose(2, 1, 0).reshape(BT, D)
        out[b_rows] += wts[heavy][CA:, None] * yb[:nb]
    return out.reshape(B, S, D)
